# revision 26
# baseline (speedup 1.0000x reference)
"""Trainium2 Bass kernel for NonlocalSingleBlock (B=8, C=256, N=2048) — v8.

Data-parallel over batch B (one element per core). Structure vs v7:
  - beta staged bf16 (halves the dominant HBM stream: 16.7MB -> 8.4MB/core)
  - x staged ONLY as bf16 (residual read + all matmul operands); all matmuls
    bf16 (1 cycle/row on PE regardless of free size)
  - V1T matmul trimmed 256 -> 132 free cols (bf16 has no >=256 f32r rule);
    col 128 carries g, cols 129-131 pad
  - exp merged: Act processes 4 chunks per instruction ([128, 4, 512])
  - softmax sums: es chunk-pairs pre-added on DVE (bf16, 2x/4x mode), halving
    the ones-matmul PE rows (32768 -> 16384)
  - (st+g)*beta stays on DVE (walrus: GPSIMD cannot access PSUM); the
    SBUF-only es pair-adds go to GPSIMD(Pool) instead
  - V1T bias folded algebraically: msg = (wv1 x)@es + bv1*sums, so bv1
    rides the h1 activation bias; V1T tiles are bare PSUM->SBUF copies (Act)
  - PE stream software-pipelined: score matmuls of chunk-group k+1 are
    emitted before msg/sums matmuls of group k so PE never waits on the
    DVE -> Act round trip.
Softmax stays max-free (es in bf16; |S*beta| < 88). PSUM f32 accumulate.
"""

import numpy as np
import ml_dtypes

import concourse.bass as bass
import concourse.bacc as bacc
import concourse.tile as tile
import concourse.mybir as mybir
import concourse.bass_utils as bass_utils

B, C, N = 8, 256, 2048
EPS = 1e-5
F32 = mybir.dt.float32
BF16 = mybir.dt.bfloat16
NB = 4          # n-blocks per core
BLK = N // NB   # 512 query columns per block
MCH = N // 128  # 16 key chunks of 128
MG = 4          # chunks per exp group
NGRP = MCH // MG

_CACHE = {}


def _pack_layout():
    """Column layout of packed weight images: (lay4 f32, lay16 bf16)."""
    entries4 = [("biasK", 2), ("bias1", 4), ("c0", 1)]
    entries16 = [("Atil", 512), ("wv1g", 260), ("w2T", 128), ("w3T", 256),
                 ("ones", 128)]
    lay4, n4 = {}, 0
    for name, ncols in entries4:
        lay4[name] = (n4, ncols)
        n4 += ncols
    lay16, n16 = {}, 0
    for name, ncols in entries16:
        lay16[name] = (n16, ncols)
        n16 += ncols
    return lay4, lay16, n4, n16


def build_nc(loop_iters=None):
    nc = bacc.Bacc("TRN2", target_bir_lowering=False, debug=False)

    d = {}
    d["x16"] = nc.dram_tensor("x16", [C, N], BF16, kind="ExternalInput")
    d["betaT"] = nc.dram_tensor("betaT", [N, N], BF16, kind="ExternalInput")
    lay4, lay16, n4, n16 = _pack_layout()
    d["wpack"] = nc.dram_tensor("wpack", [128, n4], F32, kind="ExternalInput")
    d["wpack16"] = nc.dram_tensor("wpack16", [128, n16], BF16,
                                  kind="ExternalInput")
    d["out"] = nc.dram_tensor("out", [C, N], F32, kind="ExternalOutput")

    from contextlib import ExitStack, nullcontext
    with tile.TileContext(nc) as tc, ExitStack() as ctx:
        P = {}
        P["consts"] = ctx.enter_context(tc.tile_pool(name="consts", bufs=1))
        P["big"] = ctx.enter_context(tc.tile_pool(name="big", bufs=1))
        P["bt"] = ctx.enter_context(tc.tile_pool(name="bt", bufs=12))
        P["sbm"] = ctx.enter_context(tc.tile_pool(name="sbm", bufs=3))
        P["es"] = ctx.enter_context(tc.tile_pool(name="es", bufs=3))
        P["ep"] = ctx.enter_context(tc.tile_pool(name="ep", bufs=6))
        P["recip"] = ctx.enter_context(tc.tile_pool(name="recip", bufs=2))
        P["h"] = ctx.enter_context(tc.tile_pool(name="h", bufs=4))
        P["outp"] = ctx.enter_context(tc.tile_pool(name="outp", bufs=4))
        # PSUM banks: st x4 + msg + sums + proj x2 = 8
        P["st"] = ctx.enter_context(tc.tile_pool(name="st", bufs=4, space="PSUM"))
        P["sums"] = ctx.enter_context(tc.tile_pool(name="sums", bufs=1, space="PSUM"))
        P["msg"] = ctx.enter_context(tc.tile_pool(name="msg", bufs=1, space="PSUM"))
        P["proj"] = ctx.enter_context(tc.tile_pool(name="proj", bufs=2, space="PSUM"))

        cst = _load_consts(nc, P, d)
        loop_cm = tc.For_i(0, loop_iters, 1) if loop_iters else nullcontext()
        with loop_cm:
            _emit_body(nc, tc, P, d, cst)

    nc.compile()
    return nc


def _load_consts(nc, P, d):
    consts = P["consts"]
    lay4, lay16, n4, n16 = _pack_layout()
    cst = {}
    # wp16 (Atil) first: it gates the first real matmul
    wp16 = consts.tile([128, n16], BF16, name="wp16_sb")
    nc.sync.dma_start(out=wp16, in_=d["wpack16"].ap())
    wp4 = consts.tile([128, n4], F32, name="wp4_sb")
    nc.sync.dma_start(out=wp4, in_=d["wpack"].ap())

    def sl(name):
        lay, t = (lay4, wp4) if name in lay4 else (lay16, wp16)
        off, ncols = lay[name]
        return t[:, off:off + ncols]

    cst["Atil"] = sl("Atil").rearrange("p (t o) -> p t o", t=2)
    cst["wv1g"] = sl("wv1g").rearrange("p (t o) -> p t o", t=2)  # [128,2,130]
    cst["biasK"] = sl("biasK")                        # [128,2] Ktil bias
    b1 = sl("bias1")
    cst["b1"] = b1[:, 0:1]
    cst["b2"] = b1[:, 1:2]
    cst["b3"] = b1[:, 2:4]
    cst["c0"] = sl("c0")                              # [128,1] bq.bk replicated
    cst["w2T"] = sl("w2T")
    cst["w3T"] = sl("w3T")
    cst["ones"] = sl("ones")
    return cst


def _emit_body(nc, tc, P, d, cst):
    AF = mybir.ActivationFunctionType
    OP = mybir.AluOpType
    betaT_d, out_d = d["betaT"], d["out"]

    # ---- PE p-state warm-up: the tensor engine reaches 2.4GHz only after
    # ~3us of continuous execution. Burn dummy matmuls on a zeroed tile
    # during the initial DMA wait so real work starts at full clock. ----
    warm = P["big"].tile([2, 512], BF16, tag="warm", name="warm_sb")
    nc.vector.memset(warm, 0.0)
    for w in range(12):
        wp = P["proj"].tile([2, 512], F32, tag="proj", name="warm_ps")
        nc.tensor.matmul(wp, warm[0:2, 0:2], warm, start=True, stop=True)

    # ---- x (bf16: matmul operands + residual) ----
    x16_sb = P["big"].tile([128, 2, N], BF16, tag="x16", name="x16_sb")
    x16_re = d["x16"].ap().rearrange("(t p) n -> p t n", p=128)
    for q in range(4):
        qs = slice(q * (N // 4), (q + 1) * (N // 4))
        nc.sync.dma_start(out=x16_sb[:, :, qs], in_=x16_re[:, :, qs])

    kt_sb = P["big"].tile([128, 2, N], BF16, tag="kt", name="kt_sb")
    v1t_sb = P["big"].tile([128, MCH, 128], BF16, tag="v1t", name="v1t_sb")
    g_sb = P["big"].tile([128, MCH, 1], F32, tag="g", name="g_sb")

    def emit_ktil_quarter(q):
        # Ktil[d, m-quarter] = sum_c Atil[d, c] x[c, m] + biasK[d]
        nsl = slice(q * BLK, (q + 1) * BLK)
        for co in range(2):
            ps = P["proj"].tile([128, BLK], F32, tag="proj", name="kt_ps")
            for ci in range(2):
                nc.tensor.matmul(
                    ps, cst["Atil"][:, ci, co * 128:(co + 1) * 128],
                    x16_sb[:, ci, nsl], start=(ci == 0), stop=(ci == 1))
            if co == 0:
                nc.scalar.add(kt_sb[:, co, nsl], ps, cst["biasK"][:, co:co + 1])
            else:
                nc.vector.tensor_scalar_add(kt_sb[:, co, nsl], ps,
                                            cst["biasK"][:, co:co + 1])

    def emit_v1t_quad(q):
        # V1T[m, j] = sum_c x[c, m] wv1g[c, j]; col 128 is g (bv1 folded
        # into b1; c0 added on DVE)
        for mi in range(q * MG, (q + 1) * MG):
            ps = P["st"].tile([128, 130], F32, tag="st", name="v1t_ps")
            for ci in range(2):
                nc.tensor.matmul(
                    ps, x16_sb[:, ci, mi * 128:(mi + 1) * 128],
                    cst["wv1g"][:, ci, :],
                    start=(ci == 0), stop=(ci == 1))
            nc.scalar.copy(v1t_sb[:, mi, :], ps[:, 0:128])
            nc.vector.tensor_add(g_sb[:, mi, :], ps[:, 128:129], cst["c0"])

    # ---- attention + MLP: globally software-pipelined over 16 groups ----
    S = {}  # per-block state: bts, msg_ps, sums_ps; per-group es4

    def emit_bt_prefetch(nb):
        nsl = slice(nb * BLK, (nb + 1) * BLK)
        bts = {}
        for mp in range(MCH // 2):
            bt = P["bt"].tile([128, 2, BLK], BF16, tag="bt", name="bt_sb")
            nc.sync.dma_start(
                out=bt,
                in_=betaT_d.ap()[2 * mp * 128:(2 * mp + 2) * 128, nsl]
                    .rearrange("(a p) n -> p a n", p=128))
            bts[mp] = bt
        S[("bt", nb)] = bts

    # group descriptors: (nb, chunk_lo, n_chunks); the final block runs in
    # half-size groups so the end-of-kernel pipeline drain is shorter
    groups = []
    for nb in range(NB):
        mg = 2 if nb == NB - 1 else MG
        for k in range(MCH // mg):
            groups.append((nb, k * mg, mg))

    def emit_scores(gi):
        nb, clo, mg = groups[gi]
        nsl = slice(nb * BLK, (nb + 1) * BLK)
        bts = S[("bt", nb)]
        sbm4 = P["sbm"].tile([128, mg, BLK], F32, tag="sbm", name="sbm_sb")
        es4 = P["es"].tile([128, mg, BLK], BF16, tag="es", name="es_sb")
        sts = []
        for j in range(mg):
            mi = clo + j
            msl = slice(mi * 128, (mi + 1) * 128)
            st = P["st"].tile([128, BLK], F32, tag="st", name="st_ps")
            for ci in range(2):
                nc.tensor.matmul(
                    st, kt_sb[:, ci, msl], x16_sb[:, ci, nsl],
                    start=(ci == 0), stop=(ci == 1))
            sts.append(st)
        for j in range(mg):
            mi = clo + j
            nc.vector.scalar_tensor_tensor(
                out=sbm4[:, j, :], in0=sts[j], scalar=g_sb[:, mi, 0:1],
                in1=bts[mi // 2][:, mi % 2, :], op0=OP.add, op1=OP.mult)
        nc.scalar.activation(es4, sbm4, AF.Exp)
        S[("es", gi)] = es4

    def ensure_acc(nb):
        if ("msg", nb) not in S:
            S[("msg", nb)] = P["msg"].tile([128, BLK], F32, tag="msg",
                                           name="msg_ps")
            S[("sums", nb)] = P["sums"].tile([128, BLK], F32, tag="sums",
                                             name="sums_ps")

    def emit_consume_msg(gi):
        nb, clo, mg = groups[gi]
        es4 = S[("es", gi)]
        ensure_acc(nb)
        msg_ps = S[("msg", nb)]
        for j in range(mg):
            mi = clo + j
            nc.tensor.matmul(msg_ps, v1t_sb[:, mi, :], es4[:, j, :],
                             start=(mi == 0), stop=(mi == MCH - 1))
        # kick the pair-adds now (Pool for blocks 0-2; DVE on the last block
        # where Pool's ~1.1us latency would sit on the exposed drain path)
        eps = []
        for pj in range(mg // 2):
            ep = P["ep"].tile([128, BLK], BF16, tag="ep", name="ep_sb")
            # DVE only for the last 2 groups of the final block, where
            # Pool's ~1.1us latency would sit on the exposed drain path
            eng = nc.vector if (nb == NB - 1 and clo >= MCH - 4) else nc.gpsimd
            eng.tensor_add(ep, es4[:, 2 * pj, :], es4[:, 2 * pj + 1, :])
            eps.append(ep)
        S[("ep", gi)] = eps

    def emit_consume_sums(gi):
        nb, clo, mg = groups[gi]
        S.pop(("es", gi))
        ensure_acc(nb)
        sums_ps = S[("sums", nb)]
        for pj, ep in enumerate(S.pop(("ep", gi))):
            pi = (clo // 2) + pj
            nc.tensor.matmul(sums_ps, cst["ones"], ep,
                             start=(pi == 0), stop=(pi == MCH // 2 - 1))

    def emit_tail_a(nb):
        # recip/mnorm/h1 go FIRST in the iteration so they sit early in the
        # DVE/Act in-order streams (their deps resolved a full group ago)
        msg_ps, sums_ps = S.pop(("msg", nb)), S.pop(("sums", nb))
        recip = P["recip"].tile([128, BLK], F32, tag="recip", name="recip_sb")
        nc.vector.reciprocal(recip, sums_ps)
        # h1 = relu(msg1 * recip + b1f + bv1)
        mnorm = P["h"].tile([128, BLK], F32, tag="mn", name="mn_sb")
        nc.vector.tensor_mul(mnorm, msg_ps, recip)
        h1 = P["h"].tile([128, BLK], BF16, tag="h1", name="h1_sb")
        nc.scalar.activation(h1, mnorm, AF.Relu, bias=cst["b1"][:, 0:1])
        S[("h1", nb)] = h1
        S[("sumsfree", nb)] = sums_ps

    def emit_tail_b(nb):
        nsl = slice(nb * BLK, (nb + 1) * BLK)
        h1 = S.pop(("h1", nb))
        h2p = P["proj"].tile([128, BLK], F32, tag="proj", name="h2_ps")
        nc.tensor.matmul(h2p, cst["w2T"], h1, start=True, stop=True)
        h2 = P["h"].tile([128, BLK], BF16, tag="h2", name="h2_sb")
        nc.scalar.activation(h2, h2p, AF.Relu, bias=cst["b2"][:, 0:1])
        for co in range(2):
            h3p = P["proj"].tile([128, BLK], F32, tag="proj", name="h3_ps")
            nc.tensor.matmul(h3p, cst["w3T"][:, co * 128:(co + 1) * 128],
                             h2, start=True, stop=True)
            ob = P["outp"].tile([128, BLK], F32, tag="ob", name="ob_sb")
            nc.vector.scalar_tensor_tensor(
                out=ob, in0=h3p, scalar=cst["b3"][:, co:co + 1],
                in1=x16_sb[:, co, nsl], op0=OP.add, op1=OP.add)
            nc.sync.dma_start(
                out=out_d.ap()[co * 128:(co + 1) * 128, nsl], in_=ob)

    # schedule: prologue quarters woven into block 0; scores one group
    # ahead of consume; tails split and lagged 2 groups behind their block
    NG = len(groups)
    block_end = {}
    for i, (nb, clo, mg) in enumerate(groups):
        block_end[nb] = i
    tail_at = {block_end[nb] + 2: nb for nb in range(NB)}

    def sums_lag(gi):
        # blocks 0-2: sums one group behind msg (Pool latency slack);
        # last block: immediate (short drain)
        return 2 if groups[gi][0] < NB - 1 else 1

    emit_bt_prefetch(0)
    done_sums = 0
    for gi in range(NG):
        nb, clo, mg = groups[gi]
        if clo == 0 and nb > 0:
            emit_bt_prefetch(nb)
        if gi < 4:
            emit_ktil_quarter(gi)
            emit_v1t_quad(gi)
        emit_scores(gi)
        if gi >= 1:
            emit_consume_msg(gi - 1)
        # sums for any group whose lag has expired (always after its msg)
        while done_sums < gi and done_sums <= gi - sums_lag(done_sums):
            emit_consume_sums(done_sums)
            done_sums += 1
        if gi in tail_at:
            emit_tail_a(tail_at[gi])
            emit_tail_b(tail_at[gi])
    emit_consume_msg(NG - 1)
    while done_sums < NG:
        emit_consume_sums(done_sums)
        done_sums += 1
    emit_tail_a(NB - 1)
    emit_tail_b(NB - 1)


def _prep_host(inputs):
    f = np.float32
    bf = ml_dtypes.bfloat16
    wq, bq = np.asarray(inputs["wq"], f), np.asarray(inputs["bq"], f)
    wk, bk = np.asarray(inputs["wk"], f), np.asarray(inputs["bk"], f)
    wv, bv = np.asarray(inputs["wv"], f), np.asarray(inputs["bv"], f)
    inv1 = inputs["g1"] / np.sqrt(inputs["v1"] + EPS)
    w1f = (np.asarray(inputs["w1"], f) * inv1[:, None].astype(f))
    b1f = (inputs["b1"] * inv1 + inputs["be1"] - inputs["m1"] * inv1).astype(f)
    inv2 = inputs["g2"] / np.sqrt(inputs["v2"] + EPS)
    w2f = (np.asarray(inputs["w2"], f) * inv2[:, None].astype(f))
    b2f = (inputs["b2"] * inv2 + inputs["be2"] - inputs["m2"] * inv2).astype(f)
    w3, b3 = np.asarray(inputs["w3"], f), np.asarray(inputs["b3"], f)

    # folded operators
    Atil = wq.T @ wk                    # [256,256]
    biasK = wq.T @ bk                   # [256]
    u = wk.T @ bq                       # [256] -> g via V1T col 128
    c0 = float(bq @ bk)
    wv1 = w1f @ wv                      # [128,256]
    bv1 = w1f @ bv                      # [128]

    def fold2(wT):  # [256, X] -> [128, 2, X] -> [128, 2*X]
        X = wT.shape[1]
        return wT.reshape(2, 128, X).transpose(1, 0, 2).reshape(128, 2 * X)

    lay4, lay16, n4, n16 = _pack_layout()
    pack4 = np.zeros((128, n4), dtype=f)
    pack16 = np.zeros((128, n16), dtype=bf)

    def put4(name, arr):
        off, ncols = lay4[name]
        pack4[:, off:off + ncols] = arr

    def put16(name, arr):
        off, ncols = lay16[name]
        pack16[:, off:off + ncols] = arr.astype(bf)

    put16("Atil", fold2(Atil.T))        # lhsT[c, d] = Atil[d, c]
    wv1g = np.concatenate([wv1.T, u[:, None],
                           np.zeros((256, 1), f)], axis=1)  # [256, 130]
    put16("wv1g", fold2(wv1g))
    put4("biasK", biasK.reshape(2, 128).T)
    bias1 = np.zeros((128, 4), dtype=f)
    bias1[:, 0] = b1f + bv1             # bv1*sums/sums rides the h1 bias
    bias1[:, 1] = b2f
    bias1[:, 2:4] = b3.reshape(2, 128).T
    put4("bias1", bias1)
    put4("c0", np.full((128, 1), c0, dtype=f))
    put16("w2T", w2f.T)
    put16("w3T", w3.T)
    put16("ones", np.ones((128, 128), dtype=f))

    x = np.asarray(inputs["cors_feature"], dtype=f)
    beta = np.asarray(inputs["beta_attention"], dtype=f)
    shared = {"wpack": pack4, "wpack16": pack16}
    in_maps = []
    for b in range(B):
        m = dict(shared)
        m["x16"] = np.ascontiguousarray(x[b]).astype(bf)
        m["betaT"] = np.ascontiguousarray(beta[b].T).astype(bf)
        in_maps.append(m)
    return in_maps


def kernel(**inputs) -> np.ndarray:
    if "nc" not in _CACHE:
        _CACHE["nc"] = build_nc()
    nc = _CACHE["nc"]
    in_maps = _prep_host(inputs)
    res = bass_utils.run_bass_kernel_spmd(
        nc, in_maps, core_ids=list(range(B)), trace=False)
    out = np.stack([res.results[b]["out"] for b in range(B)], axis=0)
    return out.astype(np.float32)
